# revision 6
# baseline (speedup 1.0000x reference)
"""Distributed ring-attention kernel for Trainium2 (8 NeuronCores, Bass/Tile).

Strategy (seq-parallel attention, full softmax without max-subtraction):
  - Host: transpose/cast inputs to bf16; shard x.T column-wise (seq) across 8 cores.
  - Per core: project Q/K/V for its 512-seq shard; AllGather K^T and V
    (bf16) across cores; compute full attention for its Q shard over the
    whole 4096-length K/V; apply out-projection; write its y shard.
  - Scores are computed transposed (S^T = K @ Q^T, kpos on partitions) so the
    exp'd probabilities feed the P@V matmul directly as the stationary-side
    contraction. Softmax denominator comes for free from a ones-column
    appended to V. Softmax skips max-subtraction: scores are O(1) here
    (exp is numerically safe), which matches softmax exactly in exact math.
"""

import numpy as np
import ml_dtypes

HID = 1024
HEADS = 16
HD = 64
S = 4096
NCORES = 8
SQ = S // NCORES          # 512 q rows per core
PAIRS = HEADS // 2        # 8 head pairs (128 rows of qkvT per pair)
KTILES = S // 128         # 32 kpos tiles per head
VAUG = HD + 1             # 65: V plus ones column
SCALE = 1.0 / np.sqrt(HD)

_cache = {}


def _build():
    import concourse.bass as bass
    import concourse.mybir as mybir
    import concourse.tile as tile
    from concourse import bacc

    dt = mybir.dt
    nc = bacc.Bacc("TRN2", target_bir_lowering=False, debug=False,
                   num_devices=NCORES)

    xT = nc.dram_tensor("xT", [HID, SQ], dt.bfloat16, kind="ExternalInput").ap()
    wqkvT = nc.dram_tensor("wqkvT", [HID, 3 * HID], dt.bfloat16,
                           kind="ExternalInput").ap()
    woutT = nc.dram_tensor("woutT", [HID, HID], dt.bfloat16,
                           kind="ExternalInput").ap()
    y = nc.dram_tensor("y", [SQ, HID], dt.float32, kind="ExternalOutput").ap()

    with tile.TileContext(nc) as tc:
        _body(nc, tc, bass, mybir, xT, wqkvT, woutT, y)

    nc.compile()
    return nc


def _body(nc, tc, bass, mybir, xT, wqkvT, woutT, y):
    dt = mybir.dt
    f32, bf16 = dt.float32, dt.bfloat16

    with (
        tc.tile_pool(name="dram", bufs=1, space="DRAM") as dram,
        tc.tile_pool(name="resident", bufs=1) as res,
        tc.tile_pool(name="stream", bufs=1) as st,
    ):
        # ---- DRAM bounce buffers for collectives ----
        kt_bounce = dram.tile([HID, SQ], bf16, name="kt_bounce")
        v_bounce = dram.tile([SQ, HEADS * VAUG], bf16, name="v_bounce")
        ktg = dram.tile([NCORES * HID, SQ], bf16, addr_space="Shared",
                        name="ktg")
        vg = dram.tile([S, HEADS * VAUG], bf16, addr_space="Shared", name="vg")

        # ---- load xT (hidden x local-seq), 8 resident tiles ----
        xt = []
        for k in range(8):
            t = res.tile([128, SQ], bf16, tag=f"xt{k}", name=f"xt{k}")
            nc.sync.dma_start(t[:], xT[k * 128:(k + 1) * 128, :])
            xt.append(t)

        # ones columns for the V augmentation (written into v_bounce)
        ones_sb = res.tile([128, HEADS], bf16, tag="ones", name="ones_sb")
        nc.vector.memset(ones_sb[:], 1.0)

        with tc.tile_pool(name="psP", bufs=1, space="PSUM") as psP:
            # ---- K^T projection: kT[d_row, s] rows 1024..2047 of qkvT ----
            for m in range(8):
                ps = psP.tile([128, SQ], f32, tag="proj", bufs=2)
                for k in range(8):
                    w = st.tile([128, 128], bf16, tag="wl", bufs=4)
                    nc.sync.dma_start(
                        w[:], wqkvT[k * 128:(k + 1) * 128,
                                    HID + m * 128:HID + (m + 1) * 128])
                    nc.tensor.matmul(ps[:], w[:], xt[k][:],
                                     start=(k == 0), stop=(k == 7))
                sb = st.tile([128, SQ], bf16, tag="kv_stage", bufs=3)
                nc.vector.tensor_copy(sb[:], ps[:])
                nc.sync.dma_start(kt_bounce[m * 128:(m + 1) * 128, :], sb[:])

            # ---- V projection in natural [s, d] layout into aug columns ----
            # v_bounce cols: head h occupies [h*65, h*65+65) as [V(64) | 1];
            # the ones column makes the P@V matmul emit the softmax
            # denominator on psum partition 64 for free.
            for sti in range(4):
                for nch in range(2):   # output dim chunk of 512 (8 heads)
                    ps = psP.tile([128, 512], f32, tag="proj", bufs=2)
                    for k in range(8):
                        wv = st.tile([128, 512], bf16, tag="wv", bufs=3)
                        nc.sync.dma_start(
                            wv[:], wqkvT[k * 128:(k + 1) * 128,
                                         2 * HID + nch * 512:
                                         2 * HID + (nch + 1) * 512])
                        nc.tensor.matmul(
                            ps[:], xt[k][:, sti * 128:(sti + 1) * 128], wv[:],
                            start=(k == 0), stop=(k == 7))
                    sb = st.tile([128, 512], bf16, tag="kv_stage", bufs=3)
                    nc.vector.tensor_copy(sb[:], ps[:])
                    # scatter the 8 heads of this chunk into aug layout
                    rows3 = v_bounce[sti * 128:(sti + 1) * 128, :].rearrange(
                        "p (h c) -> p h c", c=VAUG)
                    sb3 = sb.rearrange("p (h c) -> p h c", c=HD)
                    nc.sync.dma_start(
                        rows3[:, nch * 8:(nch + 1) * 8, 0:HD], sb3[:])
                # ones columns for all 16 heads of this row block
                rows3 = v_bounce[sti * 128:(sti + 1) * 128, :].rearrange(
                    "p (h c) -> p h c", c=VAUG)
                nc.sync.dma_start(rows3[:, :, HD:HD + 1], ones_sb[:, :, None])

            # ---- AllGather K^T and V(aug) ----
            nc.gpsimd.collective_compute(
                "AllGather", mybir.AluOpType.bypass,
                replica_groups=[list(range(NCORES))],
                ins=[kt_bounce.opt()], outs=[ktg.opt()])
            nc.gpsimd.collective_compute(
                "AllGather", mybir.AluOpType.bypass,
                replica_groups=[list(range(NCORES))],
                ins=[v_bounce.opt()], outs=[vg.opt()])

            # ---- Q^T projection (rows 0..1023 of qkvT), resident ----
            qt = []
            for m in range(PAIRS):
                ps = psP.tile([128, SQ], f32, tag="proj", bufs=2)
                for k in range(8):
                    w = st.tile([128, 128], bf16, tag="wl", bufs=4)
                    nc.sync.dma_start(
                        w[:], wqkvT[k * 128:(k + 1) * 128,
                                    m * 128:(m + 1) * 128])
                    nc.tensor.matmul(ps[:], w[:], xt[k][:],
                                     start=(k == 0), stop=(k == 7))
                t = res.tile([128, SQ], bf16, tag=f"qt{m}", name=f"qt{m}")
                nc.vector.tensor_copy(t[:], ps[:])
                qt.append(t)

            # odd heads need their q^T at partition base 0 for the scores
            # matmul (moving operand feeds array rows 0..63)
            qlo = []
            for m in range(PAIRS):
                t = res.tile([64, SQ], bf16, tag=f"qlo{m}", name=f"qlo{m}")
                nc.sync.dma_start(t[:], qt[m][64:128, :])
                qlo.append(t)

        # ---- attention ----
        # per-head slot stream: 32 kpos tiles, exp groups of 3 psum banks
        attn = []
        for h in range(HEADS):
            t = res.tile([64, SQ], bf16, tag=f"attn{h}", name=f"attn{h}")
            attn.append(t)

        with tc.tile_pool(name="psA", bufs=1, space="PSUM") as psA:
            for h in range(HEADS):
                p, e = h // 2, h % 2
                rhs_q = qt[p][0:64, :] if e == 0 else qlo[p][:]
                pv = psA.tile([128, 512], f32, tag="pv", bufs=2)

                groups = [list(range(g, min(g + 3, KTILES)))
                          for g in range(0, KTILES, 3)]
                for gi, group in enumerate(groups):
                    gw = 512 * len(group)
                    sc = psA.tile([128, 1536], f32, tag="sc", bufs=2)
                    for idx, t in enumerate(group):
                        c, j = t // 4, t % 4
                        kt = st.tile([64, 128], bf16, tag="kt", bufs=6)
                        nc.sync.dma_start(
                            kt[:], ktg[c * HID + h * HD:c * HID + (h + 1) * HD,
                                       j * 128:(j + 1) * 128])
                        nc.tensor.matmul(sc[:, idx * 512:(idx + 1) * 512],
                                         kt[:], rhs_q, start=True, stop=True)
                    pt = st.tile([128, 1536], bf16, tag="pt", bufs=3)
                    nc.scalar.activation(pt[:, 0:gw], sc[:, 0:gw],
                                         mybir.ActivationFunctionType.Exp,
                                         scale=float(SCALE))
                    for idx, t in enumerate(group):
                        va = st.tile([128, VAUG], bf16, tag="va", bufs=6)
                        nc.sync.dma_start(
                            va[:], vg[t * 128:(t + 1) * 128,
                                      h * VAUG:(h + 1) * VAUG])
                        nc.tensor.matmul(pv[0:VAUG, :],
                                         va[:], pt[:, idx * 512:(idx + 1) * 512],
                                         start=(t == 0), stop=(t == KTILES - 1))

                # normalize: out_head = pv_data / l  (l = ones-column row 64)
                # (partition_broadcast reads the tile's partition 0, so bounce
                # the l row down to partition 0 via DMA first)
                ls = st.tile([128, 512], f32, tag="ls", bufs=2)
                nc.vector.tensor_copy(ls[64:65, :], pv[64:65, :])
                l0 = st.tile([1, 512], f32, tag="l0", bufs=2)
                nc.sync.dma_start(l0[:], ls[64:65, :])
                r0 = st.tile([1, 512], f32, tag="r0", bufs=2)
                nc.vector.reciprocal(r0[:], l0[:])
                rb = st.tile([64, 512], f32, tag="rb", bufs=2)
                nc.gpsimd.partition_broadcast(rb[:], r0[:])
                nc.vector.tensor_mul(attn[h][:], pv[0:64, :], rb[:])

        # ---- out projection: y[s, o] = sum_h attn_h^T.T @ woutT[h rows] ----
        with tc.tile_pool(name="psY", bufs=1, space="PSUM") as psY:
            wo = {}
            for h in range(HEADS):
                for och in range(2):
                    w = res.tile([64, 512], bf16, tag=f"wo{h}_{och}",
                                 name=f"wo{h}_{och}")
                    nc.sync.dma_start(
                        w[:], woutT[h * HD:(h + 1) * HD,
                                    och * 512:(och + 1) * 512])
                    wo[(h, och)] = w
            for sti in range(4):
                for och in range(2):
                    ps = psY.tile([128, 512], f32, tag="y", bufs=2)
                    for h in range(HEADS):
                        nc.tensor.matmul(
                            ps[:], attn[h][:, sti * 128:(sti + 1) * 128],
                            wo[(h, och)][:],
                            start=(h == 0), stop=(h == HEADS - 1))
                    ysb = st.tile([128, 512], f32, tag="ysb", bufs=3)
                    nc.vector.tensor_copy(ysb[:], ps[:])
                    nc.sync.dma_start(
                        y[sti * 128:(sti + 1) * 128,
                          och * 512:(och + 1) * 512], ysb[:])


def _get_nc():
    if "nc" not in _cache:
        _cache["nc"] = _build()
    return _cache["nc"]


def kernel(x, W_qkv, W_out, _trace=False):
    from concourse.bass_utils import run_bass_kernel_spmd

    nc = _get_nc()
    bf16 = ml_dtypes.bfloat16

    x = np.asarray(x)
    xTf = np.ascontiguousarray(x.reshape(S, HID).T).astype(bf16)   # [HID, S]
    wqkvT = np.ascontiguousarray(np.asarray(W_qkv).T).astype(bf16)
    woutT = np.ascontiguousarray(np.asarray(W_out).T).astype(bf16)

    in_maps = []
    for c in range(NCORES):
        in_maps.append({
            "xT": np.ascontiguousarray(xTf[:, c * SQ:(c + 1) * SQ]),
            "wqkvT": wqkvT,
            "woutT": woutT,
        })
    res = run_bass_kernel_spmd(nc, in_maps, core_ids=list(range(NCORES)),
                               trace=_trace)
    out = np.concatenate([res.results[c]["y"] for c in range(NCORES)],
                         axis=0)
    out = out.reshape(1, S, HID).astype(np.float32)
    if _trace:
        kernel.last_results = res
    return out


# revision 12
# speedup vs baseline: 1.7549x; 1.7549x over previous
"""Distributed ring-attention kernel for Trainium2 (8 NeuronCores, Bass/Tile).

Strategy (seq-parallel attention, full softmax without max-subtraction):
  - Host: transpose/cast inputs to bf16; shard x.T column-wise (seq) across 8 cores.
  - Per core: project Q/K/V for its 512-seq shard; AllGather K^T and V
    (bf16) across cores; compute full attention for its Q shard over the
    whole 4096-length K/V; apply out-projection; write its y shard.
  - Scores are computed transposed (S^T = K @ Q^T, kpos on partitions) so the
    exp'd probabilities feed the P@V matmul directly as the stationary-side
    contraction. Softmax denominator comes for free from a ones-column
    appended to V. Softmax skips max-subtraction: scores are O(1) here
    (exp is numerically safe), which matches softmax exactly in exact math.
"""

import numpy as np
import ml_dtypes

HID = 1024
HEADS = 16
HD = 64
S = 4096
NCORES = 8
SQ = S // NCORES          # 512 q rows per core
PAIRS = HEADS // 2        # 8 head pairs (128 rows of qkvT per pair)
KTILES = S // 128         # 32 kpos tiles per head
VAUG = HD + 1             # 65: V plus ones column
SCALE = 1.0 / np.sqrt(HD)

_cache = {}


def _build():
    import concourse.bass as bass
    import concourse.mybir as mybir
    import concourse.tile as tile
    from concourse import bacc

    dt = mybir.dt
    nc = bacc.Bacc("TRN2", target_bir_lowering=False, debug=False,
                   num_devices=NCORES)

    xT = nc.dram_tensor("xT", [HID, SQ], dt.bfloat16, kind="ExternalInput").ap()
    wqkvT = nc.dram_tensor("wqkvT", [HID, 3 * HID], dt.bfloat16,
                           kind="ExternalInput").ap()
    woutT = nc.dram_tensor("woutT", [HID, HID], dt.bfloat16,
                           kind="ExternalInput").ap()
    y = nc.dram_tensor("y", [SQ, HID], dt.float32, kind="ExternalOutput").ap()

    with tile.TileContext(nc) as tc:
        _body(nc, tc, bass, mybir, xT, wqkvT, woutT, y)

    nc.compile()
    return nc


def _body(nc, tc, bass, mybir, xT, wqkvT, woutT, y):
    dt = mybir.dt
    f32, bf16 = dt.float32, dt.bfloat16

    with (
        tc.tile_pool(name="dram", bufs=1, space="DRAM") as dram,
        tc.tile_pool(name="resident", bufs=1) as res,
        tc.tile_pool(name="stream", bufs=1) as st,
    ):
        # ---- DRAM bounce buffers for collectives ----
        kt_bounce = dram.tile([HID, SQ], bf16, name="kt_bounce")
        v_bounce = dram.tile([SQ, HEADS * VAUG], bf16, name="v_bounce")
        ktg = dram.tile([NCORES * HID, SQ], bf16, addr_space="Shared",
                        name="ktg")
        vg = dram.tile([S, HEADS * VAUG], bf16, addr_space="Shared", name="vg")

        # ---- load xT (hidden x local-seq), 8 resident tiles ----
        xt = []
        for k in range(8):
            t = res.tile([128, SQ], bf16, tag=f"xt{k}", name=f"xt{k}")
            nc.sync.dma_start(t[:], xT[k * 128:(k + 1) * 128, :])
            xt.append(t)

        # ones columns for the V augmentation (written into v_bounce)
        ones_sb = res.tile([128, HEADS], bf16, tag="ones", name="ones_sb")
        nc.vector.memset(ones_sb[:], 1.0)

        # wqkvT viewed as [p, m_tile, k_tile, col] for batched strip loads
        wq4 = wqkvT.rearrange("(k p) (m c) -> p m k c", p=128, c=128)

        with tc.tile_pool(name="psP", bufs=1, space="PSUM") as psP:
            # ---- K^T projection: kT[d_row, s] rows 1024..2047 of qkvT ----
            for m in range(8):
                ws = st.tile([128, 8 * 128], bf16, tag="wl", bufs=3)
                nc.sync.dma_start(
                    ws.rearrange("p (k c) -> p k c", c=128),
                    wq4[:, 8 + m, :, :])
                ps = psP.tile([128, SQ], f32, tag="proj", bufs=2)
                for k in range(8):
                    nc.tensor.matmul(ps[:], ws[:, k * 128:(k + 1) * 128],
                                     xt[k][:], start=(k == 0), stop=(k == 7))
                sb = st.tile([128, SQ], bf16, tag="kv_stage", bufs=3)
                nc.vector.tensor_copy(sb[:], ps[:])
                nc.sync.dma_start(kt_bounce[m * 128:(m + 1) * 128, :], sb[:])

            # ---- V projection in natural [s, d] layout into aug columns ----
            # v_bounce cols: head h occupies [h*65, h*65+65) as [V(64) | 1];
            # the ones column makes the P@V matmul emit the softmax
            # denominator on psum partition 64 for free.
            # V weight strips: [128, 8 k-tiles, 512] per output chunk
            wv4 = wqkvT.rearrange("(k p) (m c) -> p m k c", p=128, c=512)
            wvs = []
            for nch in range(2):
                t = res.tile([128, 8 * 512], bf16, tag=f"wvs{nch}",
                             name=f"wvs{nch}")
                nc.sync.dma_start(t.rearrange("p (k c) -> p k c", c=512),
                                  wv4[:, 4 + nch, :, :])
                wvs.append(t)
            for sti in range(4):
                for nch in range(2):   # output dim chunk of 512 (8 heads)
                    ps = psP.tile([128, 512], f32, tag="proj", bufs=2)
                    for k in range(8):
                        nc.tensor.matmul(
                            ps[:], xt[k][:, sti * 128:(sti + 1) * 128],
                            wvs[nch][:, k * 512:(k + 1) * 512],
                            start=(k == 0), stop=(k == 7))
                    sb = st.tile([128, 512], bf16, tag="kv_stage", bufs=3)
                    nc.vector.tensor_copy(sb[:], ps[:])
                    # scatter the 8 heads of this chunk into aug layout
                    rows3 = v_bounce[sti * 128:(sti + 1) * 128, :].rearrange(
                        "p (h c) -> p h c", c=VAUG)
                    sb3 = sb.rearrange("p (h c) -> p h c", c=HD)
                    nc.sync.dma_start(
                        rows3[:, nch * 8:(nch + 1) * 8, 0:HD], sb3[:])
                # ones columns for all 16 heads of this row block
                rows3 = v_bounce[sti * 128:(sti + 1) * 128, :].rearrange(
                    "p (h c) -> p h c", c=VAUG)
                nc.sync.dma_start(rows3[:, :, HD:HD + 1], ones_sb[:, :, None])

            # ---- AllGather K^T and V(aug) ----
            nc.gpsimd.collective_compute(
                "AllGather", mybir.AluOpType.bypass,
                replica_groups=[list(range(NCORES))],
                ins=[kt_bounce.opt()], outs=[ktg.opt()])
            nc.gpsimd.collective_compute(
                "AllGather", mybir.AluOpType.bypass,
                replica_groups=[list(range(NCORES))],
                ins=[v_bounce.opt()], outs=[vg.opt()])

            # ---- Q^T projection (rows 0..1023 of qkvT), resident ----
            qt = []
            for m in range(PAIRS):
                ws = st.tile([128, 8 * 128], bf16, tag="wl", bufs=3)
                nc.sync.dma_start(
                    ws.rearrange("p (k c) -> p k c", c=128),
                    wq4[:, m, :, :])
                ps = psP.tile([128, SQ], f32, tag="proj", bufs=2)
                for k in range(8):
                    nc.tensor.matmul(ps[:], ws[:, k * 128:(k + 1) * 128],
                                     xt[k][:], start=(k == 0), stop=(k == 7))
                t = res.tile([128, SQ], bf16, tag=f"qt{m}", name=f"qt{m}")
                nc.vector.tensor_copy(t[:], ps[:])
                qt.append(t)

            # odd heads need their q^T at partition base 0 for the scores
            # matmul (moving operand feeds array rows 0..63)
            qlo = []
            for m in range(PAIRS):
                t = res.tile([64, SQ], bf16, tag=f"qlo{m}", name=f"qlo{m}")
                nc.sync.dma_start(t[:], qt[m][64:128, :])
                qlo.append(t)

        # ---- attention ----
        # per-head slot stream: 32 kpos tiles, exp groups of 3 psum banks
        attn = []
        for h in range(HEADS):
            t = res.tile([64, SQ], bf16, tag=f"attn{h}", name=f"attn{h}")
            attn.append(t)

        # ktg viewed per head: element (d, chunk, col) — one strip DMA/head
        ktg4 = ktg.rearrange("(c hh d) q -> hh d c q", d=HD, hh=HEADS)
        # vg viewed per head: element (p, ktile, col)
        vg4 = vg.rearrange("(t p) (hh c) -> hh p t c", p=128, c=VAUG)

        with tc.tile_pool(name="psA", bufs=1, space="PSUM") as psA:
            for h in range(HEADS):
                p, e = h // 2, h % 2
                rhs_q = qt[p][0:64, :] if e == 0 else qlo[p][:]
                pv = psA.tile([128, 512], f32, tag="pv", bufs=2)

                # whole-head K^T strip [64, 4096] and V_aug strip [128, 32*65]
                kth = st.tile([64, S], bf16, tag="kth", bufs=2)
                nc.sync.dma_start(
                    kth.rearrange("d (c q) -> d c q", q=SQ), ktg4[h])
                vah = st.tile([128, KTILES * VAUG], bf16, tag="vah", bufs=2)
                nc.sync.dma_start(
                    vah.rearrange("p (t c) -> p t c", c=VAUG), vg4[h])

                groups = [list(range(g, min(g + 3, KTILES)))
                          for g in range(0, KTILES, 3)]
                for gi, group in enumerate(groups):
                    gw = 512 * len(group)
                    sc = psA.tile([128, 1536], f32, tag="sc", bufs=2)
                    for idx, t in enumerate(group):
                        nc.tensor.matmul(sc[:, idx * 512:(idx + 1) * 512],
                                         kth[:, t * 128:(t + 1) * 128],
                                         rhs_q, start=True, stop=True)
                    pt = st.tile([128, 1536], bf16, tag="pt", bufs=3)
                    nc.scalar.activation(pt[:, 0:gw], sc[:, 0:gw],
                                         mybir.ActivationFunctionType.Exp,
                                         scale=float(SCALE))
                    for idx, t in enumerate(group):
                        nc.tensor.matmul(pv[0:VAUG, :],
                                         vah[:, t * VAUG:(t + 1) * VAUG],
                                         pt[:, idx * 512:(idx + 1) * 512],
                                         start=(t == 0), stop=(t == KTILES - 1))

                # normalize: out_head = pv_data / l  (l = ones-column row 64)
                # (partition_broadcast reads the tile's partition 0, so bounce
                # the l row down to partition 0 via DMA first)
                ls = st.tile([128, 512], f32, tag="ls", bufs=2)
                nc.vector.tensor_copy(ls[64:65, :], pv[64:65, :])
                l0 = st.tile([1, 512], f32, tag="l0", bufs=2)
                nc.sync.dma_start(l0[:], ls[64:65, :])
                lb = st.tile([64, 512], f32, tag="lb", bufs=2)
                nc.gpsimd.partition_broadcast(lb[:], l0[:])
                rb = st.tile([64, 512], f32, tag="rb", bufs=2)
                nc.vector.reciprocal(rb[:], lb[:])
                nc.vector.tensor_mul(attn[h][:], pv[0:64, :], rb[:])

        # ---- out projection: y[s, o] = sum_h attn_h^T.T @ woutT[h rows] ----
        with tc.tile_pool(name="psY", bufs=1, space="PSUM") as psY:
            wo4 = woutT.rearrange("(hh p) (o c) -> p o hh c", p=HD, c=512)
            wo = []
            for och in range(2):
                w = res.tile([HD, HEADS * 512], bf16, tag=f"wo{och}",
                             name=f"wo{och}")
                nc.sync.dma_start(
                    w.rearrange("p (hh c) -> p hh c", c=512), wo4[:, och])
                wo.append(w)
            for sti in range(4):
                for och in range(2):
                    ps = psY.tile([128, 512], f32, tag="y", bufs=2)
                    for h in range(HEADS):
                        nc.tensor.matmul(
                            ps[:], attn[h][:, sti * 128:(sti + 1) * 128],
                            wo[och][:, h * 512:(h + 1) * 512],
                            start=(h == 0), stop=(h == HEADS - 1))
                    ysb = st.tile([128, 512], f32, tag="ysb", bufs=3)
                    nc.vector.tensor_copy(ysb[:], ps[:])
                    nc.sync.dma_start(
                        y[sti * 128:(sti + 1) * 128,
                          och * 512:(och + 1) * 512], ysb[:])


def _get_nc():
    if "nc" not in _cache:
        _cache["nc"] = _build()
    return _cache["nc"]


def kernel(x, W_qkv, W_out, _trace=False):
    from concourse.bass_utils import run_bass_kernel_spmd

    nc = _get_nc()
    bf16 = ml_dtypes.bfloat16

    x = np.asarray(x)
    xTf = np.ascontiguousarray(x.reshape(S, HID).T).astype(bf16)   # [HID, S]
    wqkvT = np.ascontiguousarray(np.asarray(W_qkv).T).astype(bf16)
    woutT = np.ascontiguousarray(np.asarray(W_out).T).astype(bf16)

    in_maps = []
    for c in range(NCORES):
        in_maps.append({
            "xT": np.ascontiguousarray(xTf[:, c * SQ:(c + 1) * SQ]),
            "wqkvT": wqkvT,
            "woutT": woutT,
        })
    res = run_bass_kernel_spmd(nc, in_maps, core_ids=list(range(NCORES)),
                               trace=_trace)
    out = np.concatenate([res.results[c]["y"] for c in range(NCORES)],
                         axis=0)
    out = out.reshape(1, S, HID).astype(np.float32)
    if _trace:
        kernel.last_results = res
    return out
